# revision 40
# baseline (speedup 1.0000x reference)
"""GPT self-attention layer (B=2, S=2048, D=1024, H=16, hd=64) on 8 TRN2 cores.

Sharding: data-parallel over batch (2) x tensor-parallel over heads (4 groups
of 4 heads). Core c handles batch b=c//4, head group g=c%4.

All matmul operands are bf16 (converted host-side); psum accumulation stays
f32, so relative error ~3e-3 against the f32 reference (tolerance 2e-2).

Per-core pipeline:
  1. x arrives pre-transposed+bf16 from the host shard step, laid out
     [p, group, dc, t] so each 512-token group is one 8KB/partition DMA
     (group 0 split per dc-chunk so the first matmul starts after 128KB).
  2. QT/KT = W.T @ x.T, bias added on DVE (tensor_scalar, per-partition);
     V = x @ Wv + bv on DVE, stored with an interleaved ones-column per head
     (softmax denominator trick).
  3. Attention per head pair, scoresT orientation [k-part, q-free]:
     scoresT = KT.T @ QT (row-tiled 64x128, both heads concurrently into one
     2-bank psum tile), pT = exp(0.125 * scoresT) on ACT (bf16 out; scalar
     engine runs ONLY Exp so the activation table never reloads), causal
     diagonal masked by DVE multiply (2x mode on bf16), psum_c[128,512] +=
     [V|1|pad].T @ pT accumulated over k-chunks (row 64 = softmax
     denominator; v padded to 128 cols/head so the weight load uses the
     compiler's automatic FWL fast path, which needs exactly 128 columns).
     Normalize: denominator (bf16) broadcast across partitions 0-63 via an
     untiled matmul against a zero-padded column-of-ones, then
     reciprocal_approx_fast on the broadcast (custom DVE ops only work at
     partition base 0 -- at base 64 they emit garbage), DVE multiply into
     per-(head, qt) bf16 ctxn tiles.
  4. Two AllToAlls (one per head pair, 1MB bf16 each) across all 8 cores;
     shards duplicated across batch halves so the program is core-uniform.
     Each pair is gathered into ctxf (dma_gather, per-core batch-offset
     row indices) immediately after its collective, BEFORE the next
     collective occupies the gpsimd queue. A tiny dummy A2A after pair 0
     keeps the CC pipeline warm, cutting pair 1's cold-start latency.
  5. out = ctx_full.T @ Wo + bo into 8 persistent psum accumulators: bias
     enters as an extra matmul against a ones-row (no DVE work), pair-0
     (even dim-chunk) matmuls run during the second AllToAll, pair-1 (odd)
     matmuls accumulate into the same psum afterwards; psum is read out via
     scalar-engine copies (DVE stays free for the attention tail) and
     DMA'd to y [512, 1024] f32.

build(stage): stage in {"proj", "attn", "a2a", "full"} for bisection; partial
stages write debug data to y instead of the final output.
"""

import contextlib
import ctypes
import sys
import types

sys.path.insert(0, "/opt/trn_rl_repo")

import ml_dtypes
import numpy as np

import concourse.bass as bass
import concourse.mybir as mybir
import concourse.tile as tile
from concourse import bacc
from concourse import bass_utils

P = 128
B, S, D = 2, 2048, 1024
NH_LOC = 4          # heads per core
HD = 64             # head dim
G = NH_LOC * HD     # local head dims = 256
MC = G // P         # m-chunks of local dims = 2
DC = D // P         # d-chunks = 8
TB = 512            # token block (output tokens per core, q-tile width)
NQT = S // TB       # q-tiles = 4
NTC = S // P        # token chunks = 16
NC = 8
NG = 4              # collective group size (batch group)

F32 = mybir.dt.float32
F32R = mybir.dt.float32r
BF16 = mybir.dt.bfloat16
Exp = mybir.ActivationFunctionType.Exp
MULT = mybir.AluOpType.mult
ADD = mybir.AluOpType.add

BF16_NP = ml_dtypes.bfloat16

_STAGES = {"proj": 1, "attn": 2, "a2a": 3, "full": 4}


def _install_ntff_hook():
    """Make trace=True work under axon: inject antenv.axon_hooks backed by
    ctypes calls into libaxon_pjrt.so (mirrors trn_agent_boot logic)."""
    if "antenv.axon_hooks" in sys.modules:
        return
    holder = {}
    mod = types.ModuleType("antenv.axon_hooks")
    mod.set_axon_ntff_profile_hook = lambda h: holder.update(h=h)
    mod.get_axon_ntff_profile_hook = lambda: holder.get("h")
    sys.modules["antenv.axon_hooks"] = mod
    try:
        lib = ctypes.CDLL("/opt/axon/libaxon_pjrt.so")
        if not hasattr(lib, "axon_start_nrt_profile"):
            return
    except OSError:
        return
    lib.axon_start_nrt_profile.argtypes = [
        ctypes.POINTER(ctypes.c_int64),
        ctypes.c_size_t,
    ]
    lib.axon_start_nrt_profile.restype = ctypes.c_int64
    lib.axon_stop_nrt_profile.argtypes = [ctypes.c_char_p]
    lib.axon_stop_nrt_profile.restype = ctypes.c_int64

    @contextlib.contextmanager
    def _hook(output_dir, device_ids):
        import jax

        jax.devices()
        if device_ids:
            ids = (ctypes.c_int64 * len(device_ids))(*device_ids)
            rc = lib.axon_start_nrt_profile(ids, len(device_ids))
        else:
            rc = lib.axon_start_nrt_profile(None, 0)
        if rc != 0:
            raise RuntimeError(f"axon_start_nrt_profile rc={rc}")
        try:
            yield
        finally:
            n = lib.axon_stop_nrt_profile(str(output_dir).encode())
            print(f"profile: {n} ntff file(s) written to {output_dir}")

    holder["h"] = _hook


def build(stage="full", coll=True):
    st = _STAGES[stage]
    nc = bacc.Bacc("TRN2", target_bir_lowering=False, debug=False, num_devices=NC)

    # xt[p, g, dc, t]: global dim d = dc*128 + p, token = g*512 + t
    xt_d = nc.dram_tensor("xt", [P, NQT, DC, TB], BF16, kind="ExternalInput").ap()
    wq_d = nc.dram_tensor("wq", [P, DC, G], BF16, kind="ExternalInput").ap()
    wk_d = nc.dram_tensor("wk", [P, DC, G], BF16, kind="ExternalInput").ap()
    wv_d = nc.dram_tensor("wv", [P, DC, G], BF16, kind="ExternalInput").ap()
    bq_d = nc.dram_tensor("bq", [P, MC], F32, kind="ExternalInput").ap()
    bk_d = nc.dram_tensor("bk", [P, MC], F32, kind="ExternalInput").ap()
    bv_d = nc.dram_tensor("bv", [1, G], F32, kind="ExternalInput").ap()
    wo_d = nc.dram_tensor("wo", [P, DC, D], BF16, kind="ExternalInput").ap()
    bo_d = nc.dram_tensor("bo", [1, D], F32, kind="ExternalInput").ap()
    gidx_d = nc.dram_tensor("gidx", [P, D // 32], mybir.dt.int16, kind="ExternalInput").ap()
    y_d = nc.dram_tensor("y", [TB, D], F32, kind="ExternalOutput").ap()

    with tile.TileContext(nc) as tc:
        with (
            tc.tile_pool(name="const", bufs=1) as const,
            tc.tile_pool(name="dram", bufs=1, space="DRAM") as dram,
            tc.tile_pool(name="persist", bufs=1) as persist,
        ):
            ps_mm = tc.alloc_tile_pool(name="ps_mm", bufs=2, space="PSUM")
            ps_ctx = tc.alloc_tile_pool(name="ps_ctx", bufs=4, space="PSUM")
            # ---------------- constants ----------------
            ones_f = const.tile([P, 1], F32, tag="ones_f")
            nc.vector.memset(ones_f[:], 1.0)
            # trimask[k, u] = 1 if k <= u else 0 (keep where u - k >= 0)
            tri_f = const.tile([P, P], F32, tag="tri_f")
            nc.gpsimd.memset(tri_f[:], 1.0)
            nc.gpsimd.affine_select(
                out=tri_f[:],
                in_=tri_f[:],
                compare_op=mybir.AluOpType.is_ge,
                fill=0.0,
                base=0,
                pattern=[[1, P]],
                channel_multiplier=-1,
            )
            tri_b = const.tile([P, P], BF16, tag="tri_b")
            nc.vector.tensor_copy(tri_b[:], tri_f[:])
            # column-of-ones at partition 64 (bf16 for the broadcast matmul)
            zrow_f = const.tile([P, P], F32, tag="zrow_f")
            nc.vector.memset(zrow_f[:], 0.0)
            nc.vector.memset(zrow_f[64:65, :], 1.0)
            onescol_b = const.tile([P, P], BF16, tag="onescol_b")
            nc.vector.tensor_copy(onescol_b[:], zrow_f[:])
            # onesrow_b[p, m] = 1 iff p == 0: broadcasts R's row 0 over M
            zrow0_f = const.tile([P, P], F32, tag="zrow0_f")
            nc.vector.memset(zrow0_f[:], 0.0)
            nc.vector.memset(zrow0_f[0:1, :], 1.0)
            onesrow_b = const.tile([P, P], BF16, tag="onesrow_b")
            nc.vector.tensor_copy(onesrow_b[:], zrow0_f[:])
            zeros_f = const.tile([P, 512], F32, tag="zeros_f")
            nc.vector.memset(zeros_f[:], 0.0)

            bq_sb = const.tile([P, MC], F32, tag="bq")
            bk_sb = const.tile([P, MC], F32, tag="bk")
            nc.sync.dma_start(bq_sb[:], bq_d)
            nc.sync.dma_start(bk_sb[:], bk_d)
            bv_row = const.tile([1, G], F32, tag="bv_row")
            nc.sync.dma_start(bv_row[:], bv_d)
            bv_bc = const.tile([P, G], F32, tag="bv_bc")
            nc.gpsimd.partition_broadcast(bv_bc[:], bv_row[:])
            bo_row = const.tile([1, D], F32, tag="bo_row")
            bo_bc = const.tile([P, D], F32, tag="bo_bc")
            bo_b2 = const.tile([P, D], BF16, tag="bo_b2")

            # persistent activations
            qT = persist.tile([P, MC, S], BF16, tag="qT")
            kT = persist.tile([P, MC, S], BF16, tag="kT")
            VW = 128  # padded per-head width (ctx lhsT loads 128 cols for FWL)
            v_sb = persist.tile([P, NTC, NH_LOC * VW], BF16, tag="v")
            wo_sb = persist.tile([P, DC, D], BF16, tag="wo")

            # zero the pad columns, then ones column (denominator trick)
            # at col 64 of each head block
            nc.vector.memset(v_sb[:], 0.0)
            v_ones_ap = v_sb[:].rearrange("p t (h c) -> p t h c", c=VW)[
                :, :, :, HD
            ]
            nc.vector.tensor_copy(
                v_ones_ap,
                ones_f[:, 0:1, None].to_broadcast((P, NTC, NH_LOC, 1)),
            )

            a2a_in = [
                dram.tile([NC * P, TB], BF16, name=f"a2ain{p}", tag=f"a2ain{p}")
                for p in range(2)
            ]
            a2a_out = [
                dram.tile([NC * P, TB], BF16, name=f"a2aout{p}", tag=f"a2aout{p}")
                for p in range(2)
            ]
            warm_in = dram.tile([NC, 64], BF16, name="warm_in", tag="warm_in")
            warm_out = dram.tile([NC, 64], BF16, name="warm_out", tag="warm_out")

            with (
                tc.tile_pool(name="xw", bufs=1) as xw,
            ):
                wq_sb = xw.tile([P, DC, G], BF16, tag="wq")
                wk_sb = xw.tile([P, DC, G], BF16, tag="wk")
                wv_sb = xw.tile([P, DC, G], BF16, tag="wv")
                nc.sync.dma_start(wq_sb[:], wq_d)

                xTg = [
                    xw.tile([P, DC, TB], BF16, tag=f"xT{g}", name=f"xT{g}")
                    for g in range(NQT)
                ]

                for g in range(NQT):
                    if g == 0:
                        # split per dc-chunk: the dc=0 matmul can start after
                        # the first 128KB lands rather than the full 1MB
                        for dc in range(DC):
                            nc.sync.dma_start(
                                xTg[g][:, dc], xt_d[:, g, dc]
                            )
                    else:
                        nc.sync.dma_start(xTg[g][:], xt_d[:, g])
                    if g == 0:
                        nc.sync.dma_start(wk_sb[:], wk_d)
                        nc.sync.dma_start(wv_sb[:], wv_d)
                    for w_sb, b_sb, out_t in ((wq_sb, bq_sb, qT), (wk_sb, bk_sb, kT)):
                        for mc_i in range(MC):
                            pj = ps_mm.tile([P, 512], F32, tag="mm")
                            for dc in range(DC):
                                nc.tensor.matmul(
                                    pj[:],
                                    w_sb[:, dc, mc_i * P : (mc_i + 1) * P],
                                    xTg[g][:, dc, :],
                                    start=(dc == 0),
                                    stop=(dc == DC - 1),
                                )
                            nc.vector.tensor_scalar(
                                out=out_t[:, mc_i, g * TB : (g + 1) * TB],
                                in0=pj[:],
                                scalar1=b_sb[:, mc_i : mc_i + 1],
                                scalar2=None,
                                op0=ADD,
                            )
                    for ti in range(4):
                        tc_i = 4 * g + ti
                        pv = ps_mm.tile([P, G], F32, tag="mm")
                        for dc in range(DC):
                            nc.tensor.matmul(
                                pv[:],
                                xTg[g][:, dc, ti * P : (ti + 1) * P],
                                wv_sb[:, dc, :],
                                start=(dc == 0),
                                stop=(dc == DC - 1),
                            )
                        v_dst = v_sb[:].rearrange("p t (h c) -> p t h c", c=VW)[
                            :, tc_i, :, 0:HD
                        ]
                        nc.vector.tensor_tensor(
                            v_dst,
                            pv[:].rearrange("p (h c) -> p h c", c=HD),
                            bv_bc[:].rearrange("p (h c) -> p h c", c=HD),
                            ADD,
                        )

            if st == 1:  # proj debug out
                with tc.tile_pool(name="dbg", bufs=2) as dbg:
                    for tc_i in range(TB // P):
                        d_sb = dbg.tile([P, D], F32, tag="dbg")
                        nc.vector.tensor_copy(d_sb[:, 0:512], qT[:, 0, 0:512])
                        nc.vector.tensor_copy(d_sb[:, 512:768], kT[:, 0, 0:256])
                        nc.vector.tensor_copy(
                            d_sb[:, 768:1024],
                            v_sb[:].rearrange("p t c -> p (t c)")[:, 0:256],
                        )
                        nc.sync.dma_start(
                            y_d[tc_i * P : (tc_i + 1) * P, :], d_sb[:]
                        )

            if st >= 2:
                # ---------- phase 3: attention ----------
                nc.sync.dma_start(wo_sb[:], wo_d)
                nc.sync.dma_start(bo_row[:], bo_d)
                nc.gpsimd.partition_broadcast(bo_bc[:], bo_row[:])
                nc.vector.tensor_copy(bo_b2[:], bo_bc[:])
                gidx_sb = const.tile([P, D // 32], mybir.dt.int16, tag="gidx")
                nc.sync.dma_start(gidx_sb[:], gidx_d)
                outp = tc.alloc_tile_pool(name="outp", bufs=1)
                # ctxf[p, pr, g, t]: global dim chunk dc = 2*g + pr
                ctxf = outp.tile([P, 2, NQT, TB], BF16, tag="ctxf")
                work = tc.alloc_tile_pool(name="att", bufs=1)
                pTp = tc.alloc_tile_pool(name="pTp", bufs=10)
                smallp = tc.alloc_tile_pool(name="smallp", bufs=2)
                ctxn = [
                    [
                        work.tile(
                            [HD, TB], BF16, tag=f"ctxn{h}_{q}", name=f"ctxn{h}_{q}"
                        )
                        for q in range(NQT)
                    ]
                    for h in range(NH_LOC)
                ]
                # rdenX row 64 holds 1/den (bf16); other rows zero (never
                # touched after init -- the broadcast matmul needs them
                # non-NaN). rrec is the f32 scratch for reciprocal_approx.
                rdenX = [
                    work.tile([P, 512], BF16, tag=f"rdenX{i}", name=f"rdenX{i}")
                    for i in range(2)
                ]
                for i in range(2):
                    nc.vector.tensor_copy(rdenX[i][:], zeros_f[:])
                v_heads = v_sb[:].rearrange("p t (h c) -> p t h c", c=VW)
                for pair in range(MC):
                    for qt in range(NQT):
                        nkc = 4 * qt + 4
                        c_ps = [
                            ps_ctx.tile([P, 512], F32, tag="ctx", name=f"cps{h01}")
                            for h01 in range(2)
                        ]
                        for kcb in range(0, nkc, 8):  # blocks of <=8 k-chunks
                            kcs = list(range(kcb, min(kcb + 8, nkc)))
                            s_tiles = {}
                            for kc in kcs:
                                j = kc - 4 * qt
                                coff = max(0, j) * P
                                s_ps = ps_mm.tile([P, 2, 512], F32, tag="mm")
                                for h01 in range(2):
                                    pb = h01 * HD
                                    nc.tensor.matmul(
                                        s_ps[:, h01, coff:512],
                                        kT[pb : pb + HD, pair, kc * P : (kc + 1) * P],
                                        qT[
                                            pb : pb + HD,
                                            pair,
                                            qt * TB + coff : (qt + 1) * TB,
                                        ],
                                        start=True,
                                        stop=True,
                                    )
                                s_tiles[kc] = (s_ps, coff)
                            p_tiles = {}
                            for kc in kcs:
                                j = kc - 4 * qt
                                s_ps, coff = s_tiles[kc]
                                pT = pTp.tile([P, 2, 512], BF16, tag="pT")
                                nc.scalar.activation(
                                    pT[:, :, coff:512],
                                    s_ps[:, :, coff:512],
                                    Exp,
                                    scale=0.125,
                                )
                                if j >= 0:
                                    nc.vector.tensor_tensor(
                                        pT[:, :, coff : coff + P],
                                        pT[:, :, coff : coff + P],
                                        tri_b[:, None, :].to_broadcast((P, 2, P)),
                                        MULT,
                                    )
                                p_tiles[kc] = (pT, coff)
                            for kc in kcs:
                                pT, coff = p_tiles[kc]
                                for h01 in range(2):
                                    h = 2 * pair + h01
                                    nc.tensor.matmul(
                                        c_ps[h01][:, coff:512],
                                        v_heads[:, kc, h, :],
                                        pT[:, h01, coff:512],
                                        start=(kc == 0),
                                        stop=(kc == nkc - 1),
                                    )
                        for h01 in range(2):
                            h = 2 * pair + h01
                            rX = rdenX[h01]
                            nc.vector.tensor_copy(
                                rX[64:65, :], c_ps[h01][64:65, :]
                            )
                            b_ps = ps_ctx.tile([P, 512], F32, tag="ctx", name="bps")
                            nc.tensor.matmul(
                                b_ps[:],
                                onescol_b[:],
                                rX[:],
                                start=True,
                                stop=True,
                            )
                            bb = smallp.tile([HD, 512], F32, tag="bb")
                            nc.vector.reciprocal_approx_fast(
                                bb[:], b_ps[0:HD, :]
                            )
                            nc.vector.tensor_tensor(
                                ctxn[h][qt][:, :],
                                c_ps[h01][0:HD, :],
                                bb[:],
                                MULT,
                            )
                        # A2A sends for this (pair, qt): dest block qt,
                        # duplicated across batch halves (program-uniform)
                        if st >= 3:
                            for sh in (qt, qt + 4):
                                for h01 in range(2):
                                    h = 2 * pair + h01
                                    nc.sync.dma_start(
                                        a2a_in[pair][
                                            sh * P
                                            + h01 * HD : sh * P
                                            + (h01 + 1) * HD,
                                            :,
                                        ],
                                        ctxn[h][qt][:, :],
                                    )

                    # collective for this head pair; gather it into ctxf
                    # immediately (before the NEXT collective occupies the
                    # gpsimd queue) so outproj-even can start early
                    if st >= 3:
                        if coll:
                            nc.gpsimd.collective_compute(
                                "AllToAll",
                                mybir.AluOpType.bypass,
                                ins=[a2a_in[pair].opt()],
                                outs=[a2a_out[pair].opt()],
                                replica_groups=[list(range(NC))],
                            )
                        gsrc = a2a_out if coll else a2a_in
                        nc.gpsimd.dma_gather(
                            out_ap=ctxf[:, pair],
                            in_ap=gsrc[pair][:],
                            idxs_ap=gidx_sb[:],
                            num_idxs=D // 2,
                            num_idxs_reg=D // 2,
                            elem_size=TB,
                        )
                        if coll and pair == 0:
                            # tiny dummy A2A keeps the CC pipeline warm so the
                            # second real A2A avoids cold-start latency
                            nc.gpsimd.collective_compute(
                                "AllToAll",
                                mybir.AluOpType.bypass,
                                ins=[warm_in[:].opt()],
                                outs=[warm_out[:].opt()],
                                replica_groups=[list(range(NC))],
                            )

                if st == 2:  # attention debug out: raw ctxn tiles (as f32)
                    with tc.tile_pool(name="dbg2", bufs=2) as dbg2:
                        for h in range(NH_LOC):
                            for q in range(NQT):
                                d_sb = dbg2.tile([HD, TB], F32, tag="dbg2")
                                nc.vector.tensor_copy(d_sb[:], ctxn[h][q][:, :])
                                out_ap = (
                                    y_d[h * P : (h + 1) * P, :]
                                    .rearrange("a b -> (a b)")
                                    .rearrange(
                                        "(p q t) -> q p t", p=HD, q=NQT
                                    )[q]
                                )
                                nc.sync.dma_start(out_ap, d_sb[:])

                smallp.release()
                pTp.release()
                work.release()
                ps_ctx.release()
                ps_mm.release()

            if st >= 3:

                if st == 3:  # a2a debug out: gathered ctxf cols 0:128 per dc
                    with tc.tile_pool(name="dbg3", bufs=2) as dbg3:
                        for tc_i in range(TB // P):
                            d_sb = dbg3.tile([P, D], F32, tag="dbg3")
                            for dc in range(DC):
                                nc.vector.tensor_copy(
                                    d_sb[:, dc * P : (dc + 1) * P],
                                    ctxf[
                                        :, dc % 2, dc // 2, tc_i * P : (tc_i + 1) * P
                                    ],
                                )
                            nc.sync.dma_start(
                                y_d[tc_i * P : (tc_i + 1) * P, :], d_sb[:]
                            )

                if st >= 4:
                    # ---------- phase 5: output projection ----------
                    # 8 persistent psum accumulators: even (pair-0) chunks +
                    # bias row first (overlap A2A#1), odd chunks accumulate
                    # into the same psum after gather-1, y DMA'd from psum.
                    ps_out = tc.alloc_tile_pool(name="ps_out", bufs=1, space="PSUM")
                    po_u = [
                        ps_out.tile([P, 512], F32, tag=f"po{u}", name=f"po{u}")
                        for u in range(8)
                    ]
                    for u in range(8):
                        tc_i, nt = u // 2, u % 2
                        nc.tensor.matmul(
                            po_u[u][:],
                            onesrow_b[:],
                            bo_b2[:, nt * 512 : (nt + 1) * 512],
                            start=True,
                            stop=False,
                        )
                        for g in range(NQT):
                            nc.tensor.matmul(
                                po_u[u][:],
                                ctxf[:, 0, g, tc_i * P : (tc_i + 1) * P],
                                wo_sb[:, 2 * g, nt * 512 : (nt + 1) * 512],
                                start=False,
                                stop=False,
                            )
                    with tc.tile_pool(name="out_pool", bufs=3) as out_pool:
                        for u in range(8):
                            tc_i, nt = u // 2, u % 2
                            for i, g in enumerate(range(NQT)):
                                nc.tensor.matmul(
                                    po_u[u][:],
                                    ctxf[:, 1, g, tc_i * P : (tc_i + 1) * P],
                                    wo_sb[:, 2 * g + 1, nt * 512 : (nt + 1) * 512],
                                    start=False,
                                    stop=(i == NQT - 1),
                                )
                            o_sb = out_pool.tile([P, 512], F32, tag="osb")
                            nc.scalar.copy(o_sb[:], po_u[u][:])
                            nc.sync.dma_start(
                                y_d[
                                    tc_i * P : (tc_i + 1) * P,
                                    nt * 512 : (nt + 1) * 512,
                                ],
                                o_sb[:],
                            )
                    ps_out.release()

                outp.release()

    nc.compile()
    return nc


_NC_CACHE = {}


def _get_nc():
    if "nc" not in _NC_CACHE:
        _NC_CACHE["nc"] = build()
    return _NC_CACHE["nc"]


def _to_bf16(a):
    return np.ascontiguousarray(a.astype(BF16_NP))


def _make_in_maps(x, Wq, bq, Wk, bk, Wv, bv, Wo, bo):
    x = np.asarray(x, np.float32)
    Wq, Wk, Wv, Wo = (np.asarray(a, np.float32) for a in (Wq, Wk, Wv, Wo))
    bq, bk, bv, bo = (np.asarray(a, np.float32) for a in (bq, bk, bv, bo))
    # xt[b][p, g, dc, t]: d = dc*128 + p, s = g*512 + t
    xts = [
        _to_bf16(
            x[b].T.reshape(DC, P, NQT, TB).transpose(1, 2, 0, 3)
        )
        for b in range(B)
    ]
    # wo[p, dc, n]: d = dc*128 + p
    wo_t = _to_bf16(Wo.reshape(DC, P, D).transpose(1, 0, 2))
    bo_r = np.ascontiguousarray(bo.reshape(1, D))
    in_maps = []
    for c in range(NC):
        b, g = c // 4, c % 4
        sl = slice(g * G, (g + 1) * G)
        gidx = (b * (D // 2) + np.arange(D // 2)).astype(np.int16)
        in_maps.append(
            {
                "xt": xts[b],
                "wq": _to_bf16(Wq[:, sl].reshape(DC, P, G).transpose(1, 0, 2)),
                "wk": _to_bf16(Wk[:, sl].reshape(DC, P, G).transpose(1, 0, 2)),
                "wv": _to_bf16(Wv[:, sl].reshape(DC, P, G).transpose(1, 0, 2)),
                "bq": np.ascontiguousarray(bq[sl].reshape(MC, P).T),
                "bk": np.ascontiguousarray(bk[sl].reshape(MC, P).T),
                "bv": np.ascontiguousarray(bv[sl].reshape(1, G)),
                "wo": wo_t,
                "bo": bo_r,
                "gidx": np.ascontiguousarray(
                    np.tile(gidx.reshape(D // 32, 16).T, (8, 1))
                ),
            }
        )
    return in_maps


def run(inputs, trace=False, tmpdir=None):
    """Run on 8 cores; returns (output [2,2048,1024], BassKernelResults)."""
    if trace:
        _install_ntff_hook()
    nc = _get_nc()
    in_maps = _make_in_maps(**inputs)
    res = bass_utils.run_bass_kernel_spmd(
        nc, in_maps, core_ids=list(range(NC)), trace=trace, tmpdir=tmpdir
    )
    out = np.empty((B, S, D), np.float32)
    for c in range(NC):
        b, g = c // 4, c % 4
        out[b, g * TB : (g + 1) * TB, :] = res.results[c]["y"]
    return out, res


def kernel(**inputs) -> np.ndarray:
    out, _ = run(inputs, trace=False)
    return out


# revision 41
# speedup vs baseline: 1.0515x; 1.0515x over previous
"""GPT self-attention layer (B=2, S=2048, D=1024, H=16, hd=64) on 8 TRN2 cores.

Sharding: data-parallel over batch (2) x tensor-parallel over heads (4 groups
of 4 heads). Core c handles batch b=c//4, head group g=c%4.

All matmul operands are bf16 (converted host-side); psum accumulation stays
f32, so relative error ~3e-3 against the f32 reference (tolerance 2e-2).

Per-core pipeline:
  1. x arrives pre-transposed+bf16 from the host shard step, laid out
     [p, group, dc, t] so each 512-token group is one 8KB/partition DMA
     (group 0 split per dc-chunk so the first matmul starts after 128KB).
  2. QT/KT = W.T @ x.T, bias added on DVE (tensor_scalar, per-partition);
     V = x @ Wv + bv on DVE, stored with an interleaved ones-column per head
     (softmax denominator trick).
  3. Attention per head pair, scoresT orientation [k-part, q-free]:
     scoresT = KT.T @ QT (row-tiled 64x128, both heads concurrently into one
     2-bank psum tile), pT = exp(0.125 * scoresT) on ACT (bf16 out; scalar
     engine runs ONLY Exp so the activation table never reloads), causal
     diagonal masked by DVE multiply (2x mode on bf16), psum_c[128,512] +=
     [V|1|pad].T @ pT accumulated over k-chunks (row 64 = softmax
     denominator; v padded to 128 cols/head so the weight load uses the
     compiler's automatic FWL fast path, which needs exactly 128 columns).
     Normalize: denominator (bf16) broadcast across partitions 0-63 via an
     untiled matmul against a zero-padded column-of-ones, then
     reciprocal_approx_fast on the broadcast (custom DVE ops only work at
     partition base 0 -- at base 64 they emit garbage), DVE multiply into
     per-(head, qt) bf16 ctxn tiles.
  4. Two AllToAlls (one per head pair, 1MB bf16 each) across all 8 cores;
     shards duplicated across batch halves so the program is core-uniform.
     Each pair is gathered into ctxf (dma_gather, per-core batch-offset
     row indices) immediately after its collective, BEFORE the next
     collective occupies the gpsimd queue. A tiny dummy A2A after pair 0
     keeps the CC pipeline warm, cutting pair 1's cold-start latency.
  5. out = ctx_full.T @ Wo + bo into 8 persistent psum accumulators: bias
     enters as an extra matmul against a ones-row (no DVE work), pair-0
     (even dim-chunk) matmuls run during the second AllToAll, pair-1 (odd)
     matmuls accumulate into the same psum afterwards; psum is read out via
     scalar-engine copies (DVE stays free for the attention tail) and
     DMA'd to y [512, 1024] f32.

build(stage): stage in {"proj", "attn", "a2a", "full"} for bisection; partial
stages write debug data to y instead of the final output.
"""

import contextlib
import ctypes
import sys
import types

sys.path.insert(0, "/opt/trn_rl_repo")

import ml_dtypes
import numpy as np

import concourse.bass as bass
import concourse.mybir as mybir
import concourse.tile as tile
from concourse import bacc
from concourse import bass_utils

P = 128
B, S, D = 2, 2048, 1024
NH_LOC = 4          # heads per core
HD = 64             # head dim
G = NH_LOC * HD     # local head dims = 256
MC = G // P         # m-chunks of local dims = 2
DC = D // P         # d-chunks = 8
TB = 512            # token block (output tokens per core, q-tile width)
NQT = S // TB       # q-tiles = 4
NTC = S // P        # token chunks = 16
NC = 8
NG = 4              # collective group size (batch group)

F32 = mybir.dt.float32
F32R = mybir.dt.float32r
BF16 = mybir.dt.bfloat16
Exp = mybir.ActivationFunctionType.Exp
MULT = mybir.AluOpType.mult
ADD = mybir.AluOpType.add

BF16_NP = ml_dtypes.bfloat16

_STAGES = {"proj": 1, "attn": 2, "a2a": 3, "full": 4}


def _install_ntff_hook():
    """Make trace=True work under axon: inject antenv.axon_hooks backed by
    ctypes calls into libaxon_pjrt.so (mirrors trn_agent_boot logic)."""
    if "antenv.axon_hooks" in sys.modules:
        return
    holder = {}
    mod = types.ModuleType("antenv.axon_hooks")
    mod.set_axon_ntff_profile_hook = lambda h: holder.update(h=h)
    mod.get_axon_ntff_profile_hook = lambda: holder.get("h")
    sys.modules["antenv.axon_hooks"] = mod
    try:
        lib = ctypes.CDLL("/opt/axon/libaxon_pjrt.so")
        if not hasattr(lib, "axon_start_nrt_profile"):
            return
    except OSError:
        return
    lib.axon_start_nrt_profile.argtypes = [
        ctypes.POINTER(ctypes.c_int64),
        ctypes.c_size_t,
    ]
    lib.axon_start_nrt_profile.restype = ctypes.c_int64
    lib.axon_stop_nrt_profile.argtypes = [ctypes.c_char_p]
    lib.axon_stop_nrt_profile.restype = ctypes.c_int64

    @contextlib.contextmanager
    def _hook(output_dir, device_ids):
        import jax

        jax.devices()
        if device_ids:
            ids = (ctypes.c_int64 * len(device_ids))(*device_ids)
            rc = lib.axon_start_nrt_profile(ids, len(device_ids))
        else:
            rc = lib.axon_start_nrt_profile(None, 0)
        if rc != 0:
            raise RuntimeError(f"axon_start_nrt_profile rc={rc}")
        try:
            yield
        finally:
            n = lib.axon_stop_nrt_profile(str(output_dir).encode())
            print(f"profile: {n} ntff file(s) written to {output_dir}")

    holder["h"] = _hook


def build(stage="full", coll=True):
    st = _STAGES[stage]
    nc = bacc.Bacc("TRN2", target_bir_lowering=False, debug=False, num_devices=NC)

    # xt[p, g, dc, t]: global dim d = dc*128 + p, token = g*512 + t
    xt_d = nc.dram_tensor("xt", [P, NQT, DC, TB], BF16, kind="ExternalInput").ap()
    wq_d = nc.dram_tensor("wq", [P, DC, G], BF16, kind="ExternalInput").ap()
    wk_d = nc.dram_tensor("wk", [P, DC, G], BF16, kind="ExternalInput").ap()
    wv_d = nc.dram_tensor("wv", [P, DC, G], BF16, kind="ExternalInput").ap()
    bq_d = nc.dram_tensor("bq", [P, MC], F32, kind="ExternalInput").ap()
    bk_d = nc.dram_tensor("bk", [P, MC], F32, kind="ExternalInput").ap()
    bv_d = nc.dram_tensor("bv", [1, G], F32, kind="ExternalInput").ap()
    wo_d = nc.dram_tensor("wo", [P, DC, D], BF16, kind="ExternalInput").ap()
    bo_d = nc.dram_tensor("bo", [1, D], F32, kind="ExternalInput").ap()
    gidx_d = nc.dram_tensor("gidx", [P, D // 32], mybir.dt.int16, kind="ExternalInput").ap()
    y_d = nc.dram_tensor("y", [TB, D], F32, kind="ExternalOutput").ap()

    with tile.TileContext(nc) as tc:
        with (
            tc.tile_pool(name="const", bufs=1) as const,
            tc.tile_pool(name="dram", bufs=1, space="DRAM") as dram,
            tc.tile_pool(name="persist", bufs=1) as persist,
        ):
            ps_mm = tc.alloc_tile_pool(name="ps_mm", bufs=2, space="PSUM")
            ps_ctx = tc.alloc_tile_pool(name="ps_ctx", bufs=4, space="PSUM")
            # ---------------- constants ----------------
            ones_f = const.tile([P, 1], F32, tag="ones_f")
            nc.vector.memset(ones_f[:], 1.0)
            # trimask[k, u] = 1 if k <= u else 0 (keep where u - k >= 0)
            tri_f = const.tile([P, P], F32, tag="tri_f")
            nc.gpsimd.memset(tri_f[:], 1.0)
            nc.gpsimd.affine_select(
                out=tri_f[:],
                in_=tri_f[:],
                compare_op=mybir.AluOpType.is_ge,
                fill=0.0,
                base=0,
                pattern=[[1, P]],
                channel_multiplier=-1,
            )
            tri_b = const.tile([P, P], BF16, tag="tri_b")
            nc.vector.tensor_copy(tri_b[:], tri_f[:])
            # column-of-ones at partition 64 (bf16 for the broadcast matmul)
            zrow_f = const.tile([P, P], F32, tag="zrow_f")
            nc.vector.memset(zrow_f[:], 0.0)
            nc.vector.memset(zrow_f[64:65, :], 1.0)
            onescol_b = const.tile([P, P], BF16, tag="onescol_b")
            nc.vector.tensor_copy(onescol_b[:], zrow_f[:])
            # onesrow_b[p, m] = 1 iff p == 0: broadcasts R's row 0 over M
            zrow0_f = const.tile([P, P], F32, tag="zrow0_f")
            nc.vector.memset(zrow0_f[:], 0.0)
            nc.vector.memset(zrow0_f[0:1, :], 1.0)
            onesrow_b = const.tile([P, P], BF16, tag="onesrow_b")
            nc.vector.tensor_copy(onesrow_b[:], zrow0_f[:])
            zeros_f = const.tile([P, 512], F32, tag="zeros_f")
            nc.vector.memset(zeros_f[:], 0.0)

            bq_sb = const.tile([P, MC], F32, tag="bq")
            bk_sb = const.tile([P, MC], F32, tag="bk")
            nc.sync.dma_start(bq_sb[:], bq_d)
            nc.sync.dma_start(bk_sb[:], bk_d)
            bv_row = const.tile([1, G], F32, tag="bv_row")
            nc.sync.dma_start(bv_row[:], bv_d)
            bv_bc = const.tile([P, G], F32, tag="bv_bc")
            nc.gpsimd.partition_broadcast(bv_bc[:], bv_row[:])
            bo_row = const.tile([1, D], F32, tag="bo_row")
            bo_bc = const.tile([P, D], F32, tag="bo_bc")
            bo_b2 = const.tile([P, D], BF16, tag="bo_b2")

            # persistent activations
            qT = persist.tile([P, MC, S], BF16, tag="qT")
            kT = persist.tile([P, MC, S], BF16, tag="kT")
            VW = 128  # padded per-head width (ctx lhsT loads 128 cols for FWL)
            v_sb = persist.tile([P, NTC, NH_LOC * VW], BF16, tag="v")
            wo_sb = persist.tile([P, DC, D], BF16, tag="wo")

            # zero the pad columns, then ones column (denominator trick)
            # at col 64 of each head block
            nc.vector.memset(v_sb[:], 0.0)
            v_ones_ap = v_sb[:].rearrange("p t (h c) -> p t h c", c=VW)[
                :, :, :, HD
            ]
            nc.vector.tensor_copy(
                v_ones_ap,
                ones_f[:, 0:1, None].to_broadcast((P, NTC, NH_LOC, 1)),
            )

            a2a_in = [
                dram.tile([NC * P, TB], BF16, name=f"a2ain{p}", tag=f"a2ain{p}")
                for p in range(2)
            ]
            a2a_out = [
                dram.tile([NC * P, TB], BF16, name=f"a2aout{p}", tag=f"a2aout{p}")
                for p in range(2)
            ]
            warm_in = dram.tile([NC, 64], BF16, name="warm_in", tag="warm_in")
            warm_out = dram.tile([NC, 64], BF16, name="warm_out", tag="warm_out")

            with (
                tc.tile_pool(name="xw", bufs=1) as xw,
            ):
                wq_sb = xw.tile([P, DC, G], BF16, tag="wq")
                wk_sb = xw.tile([P, DC, G], BF16, tag="wk")
                wv_sb = xw.tile([P, DC, G], BF16, tag="wv")
                nc.sync.dma_start(wq_sb[:], wq_d)

                xTg = [
                    xw.tile([P, DC, TB], BF16, tag=f"xT{g}", name=f"xT{g}")
                    for g in range(NQT)
                ]

                for g in range(NQT):
                    if g == 0:
                        # split per dc-chunk: the dc=0 matmul can start after
                        # the first 128KB lands rather than the full 1MB
                        for dc in range(DC):
                            nc.sync.dma_start(
                                xTg[g][:, dc], xt_d[:, g, dc]
                            )
                    else:
                        nc.sync.dma_start(xTg[g][:], xt_d[:, g])
                    if g == 0:
                        nc.sync.dma_start(wk_sb[:], wk_d)
                        nc.sync.dma_start(wv_sb[:], wv_d)
                    for w_sb, b_sb, out_t in ((wq_sb, bq_sb, qT), (wk_sb, bk_sb, kT)):
                        for mc_i in range(MC):
                            pj = ps_mm.tile([P, 512], F32, tag="mm")
                            for dc in range(DC):
                                nc.tensor.matmul(
                                    pj[:],
                                    w_sb[:, dc, mc_i * P : (mc_i + 1) * P],
                                    xTg[g][:, dc, :],
                                    start=(dc == 0),
                                    stop=(dc == DC - 1),
                                )
                            nc.vector.tensor_scalar(
                                out=out_t[:, mc_i, g * TB : (g + 1) * TB],
                                in0=pj[:],
                                scalar1=b_sb[:, mc_i : mc_i + 1],
                                scalar2=None,
                                op0=ADD,
                            )
                    for ti in range(4):
                        tc_i = 4 * g + ti
                        pv = ps_mm.tile([P, G], F32, tag="mm")
                        for dc in range(DC):
                            nc.tensor.matmul(
                                pv[:],
                                xTg[g][:, dc, ti * P : (ti + 1) * P],
                                wv_sb[:, dc, :],
                                start=(dc == 0),
                                stop=(dc == DC - 1),
                            )
                        v_dst = v_sb[:].rearrange("p t (h c) -> p t h c", c=VW)[
                            :, tc_i, :, 0:HD
                        ]
                        nc.vector.tensor_tensor(
                            v_dst,
                            pv[:].rearrange("p (h c) -> p h c", c=HD),
                            bv_bc[:].rearrange("p (h c) -> p h c", c=HD),
                            ADD,
                        )

            if st == 1:  # proj debug out
                with tc.tile_pool(name="dbg", bufs=2) as dbg:
                    for tc_i in range(TB // P):
                        d_sb = dbg.tile([P, D], F32, tag="dbg")
                        nc.vector.tensor_copy(d_sb[:, 0:512], qT[:, 0, 0:512])
                        nc.vector.tensor_copy(d_sb[:, 512:768], kT[:, 0, 0:256])
                        nc.vector.tensor_copy(
                            d_sb[:, 768:1024],
                            v_sb[:].rearrange("p t c -> p (t c)")[:, 0:256],
                        )
                        nc.sync.dma_start(
                            y_d[tc_i * P : (tc_i + 1) * P, :], d_sb[:]
                        )

            if st >= 2:
                # ---------- phase 3: attention ----------
                nc.sync.dma_start(wo_sb[:], wo_d)
                nc.sync.dma_start(bo_row[:], bo_d)
                nc.gpsimd.partition_broadcast(bo_bc[:], bo_row[:])
                nc.vector.tensor_copy(bo_b2[:], bo_bc[:])
                gidx_sb = const.tile([P, D // 32], mybir.dt.int16, tag="gidx")
                nc.sync.dma_start(gidx_sb[:], gidx_d)
                outp = tc.alloc_tile_pool(name="outp", bufs=1)
                # ctxf[p, pr, g, t]: global dim chunk dc = 2*g + pr
                ctxf = outp.tile([P, 2, NQT, TB], BF16, tag="ctxf")
                work = tc.alloc_tile_pool(name="att", bufs=1)
                pTp = tc.alloc_tile_pool(name="pTp", bufs=10)
                smallp = tc.alloc_tile_pool(name="smallp", bufs=2)
                ctxn = [
                    [
                        work.tile(
                            [HD, TB], BF16, tag=f"ctxn{h}_{q}", name=f"ctxn{h}_{q}"
                        )
                        for q in range(NQT)
                    ]
                    for h in range(NH_LOC)
                ]
                # rdenX row 64 holds 1/den (bf16); other rows zero (never
                # touched after init -- the broadcast matmul needs them
                # non-NaN). rrec is the f32 scratch for reciprocal_approx.
                rdenX = [
                    work.tile([P, 512], BF16, tag=f"rdenX{i}", name=f"rdenX{i}")
                    for i in range(2)
                ]
                for i in range(2):
                    nc.vector.tensor_copy(rdenX[i][:], zeros_f[:])
                v_heads = v_sb[:].rearrange("p t (h c) -> p t h c", c=VW)
                for pair in range(MC):
                    for qt in range(NQT):
                        nkc = 4 * qt + 4
                        c_ps = [
                            ps_ctx.tile([P, 512], F32, tag="ctx", name=f"cps{h01}")
                            for h01 in range(2)
                        ]
                        for kcb in range(0, nkc, 8):  # blocks of <=8 k-chunks
                            kcs = list(range(kcb, min(kcb + 8, nkc)))
                            s_tiles = {}
                            for kc in kcs:
                                j = kc - 4 * qt
                                coff = max(0, j) * P
                                s_ps = ps_mm.tile([P, 2, 512], F32, tag="mm")
                                for h01 in range(2):
                                    pb = h01 * HD
                                    nc.tensor.matmul(
                                        s_ps[:, h01, coff:512],
                                        kT[pb : pb + HD, pair, kc * P : (kc + 1) * P],
                                        qT[
                                            pb : pb + HD,
                                            pair,
                                            qt * TB + coff : (qt + 1) * TB,
                                        ],
                                        start=True,
                                        stop=True,
                                    )
                                s_tiles[kc] = (s_ps, coff)
                            p_tiles = {}
                            for kc in kcs:
                                j = kc - 4 * qt
                                s_ps, coff = s_tiles[kc]
                                pT = pTp.tile([P, 2, 512], BF16, tag="pT")
                                nc.scalar.activation(
                                    pT[:, :, coff:512],
                                    s_ps[:, :, coff:512],
                                    Exp,
                                    scale=0.125,
                                )
                                if j >= 0:
                                    nc.vector.tensor_tensor(
                                        pT[:, :, coff : coff + P],
                                        pT[:, :, coff : coff + P],
                                        tri_b[:, None, :].to_broadcast((P, 2, P)),
                                        MULT,
                                    )
                                p_tiles[kc] = (pT, coff)
                            for kc in kcs:
                                pT, coff = p_tiles[kc]
                                for h01 in range(2):
                                    h = 2 * pair + h01
                                    nc.tensor.matmul(
                                        c_ps[h01][:, coff:512],
                                        v_heads[:, kc, h, :],
                                        pT[:, h01, coff:512],
                                        start=(kc == 0),
                                        stop=(kc == nkc - 1),
                                    )
                        for h01 in range(2):
                            h = 2 * pair + h01
                            rX = rdenX[h01]
                            nc.vector.tensor_copy(
                                rX[64:65, :], c_ps[h01][64:65, :]
                            )
                            b_ps = ps_ctx.tile([P, 512], F32, tag="ctx", name="bps")
                            nc.tensor.matmul(
                                b_ps[:],
                                onescol_b[:],
                                rX[:],
                                start=True,
                                stop=True,
                            )
                            bb = smallp.tile([HD, 512], F32, tag="bb")
                            nc.vector.reciprocal_approx_fast(
                                bb[:], b_ps[0:HD, :]
                            )
                            nc.vector.tensor_tensor(
                                ctxn[h][qt][:, :],
                                c_ps[h01][0:HD, :],
                                bb[:],
                                MULT,
                            )
                        # A2A sends for this (pair, qt): dest block qt,
                        # duplicated across batch halves (program-uniform)
                        if st >= 3:
                            for sh in (qt, qt + 4):
                                for h01 in range(2):
                                    h = 2 * pair + h01
                                    nc.sync.dma_start(
                                        a2a_in[pair][
                                            sh * P
                                            + h01 * HD : sh * P
                                            + (h01 + 1) * HD,
                                            :,
                                        ],
                                        ctxn[h][qt][:, :],
                                    )

                    # collective for this head pair; gather it into ctxf
                    # immediately (before the NEXT collective occupies the
                    # gpsimd queue) so outproj-even can start early
                    if st >= 3:
                        if coll:
                            nc.gpsimd.collective_compute(
                                "AllToAll",
                                mybir.AluOpType.bypass,
                                ins=[a2a_in[pair].opt()],
                                outs=[a2a_out[pair].opt()],
                                replica_groups=[list(range(NC))],
                            )
                        gsrc = a2a_out if coll else a2a_in
                        nc.gpsimd.dma_gather(
                            out_ap=ctxf[:, pair],
                            in_ap=gsrc[pair][:],
                            idxs_ap=gidx_sb[:],
                            num_idxs=D // 2,
                            num_idxs_reg=D // 2,
                            elem_size=TB,
                        )


                if st == 2:  # attention debug out: raw ctxn tiles (as f32)
                    with tc.tile_pool(name="dbg2", bufs=2) as dbg2:
                        for h in range(NH_LOC):
                            for q in range(NQT):
                                d_sb = dbg2.tile([HD, TB], F32, tag="dbg2")
                                nc.vector.tensor_copy(d_sb[:], ctxn[h][q][:, :])
                                out_ap = (
                                    y_d[h * P : (h + 1) * P, :]
                                    .rearrange("a b -> (a b)")
                                    .rearrange(
                                        "(p q t) -> q p t", p=HD, q=NQT
                                    )[q]
                                )
                                nc.sync.dma_start(out_ap, d_sb[:])

                smallp.release()
                pTp.release()
                work.release()
                ps_ctx.release()
                ps_mm.release()

            if st >= 3:

                if st == 3:  # a2a debug out: gathered ctxf cols 0:128 per dc
                    with tc.tile_pool(name="dbg3", bufs=2) as dbg3:
                        for tc_i in range(TB // P):
                            d_sb = dbg3.tile([P, D], F32, tag="dbg3")
                            for dc in range(DC):
                                nc.vector.tensor_copy(
                                    d_sb[:, dc * P : (dc + 1) * P],
                                    ctxf[
                                        :, dc % 2, dc // 2, tc_i * P : (tc_i + 1) * P
                                    ],
                                )
                            nc.sync.dma_start(
                                y_d[tc_i * P : (tc_i + 1) * P, :], d_sb[:]
                            )

                if st >= 4:
                    # ---------- phase 5: output projection ----------
                    # 8 persistent psum accumulators: even (pair-0) chunks +
                    # bias row first (overlap A2A#1), odd chunks accumulate
                    # into the same psum after gather-1, y DMA'd from psum.
                    ps_out = tc.alloc_tile_pool(name="ps_out", bufs=1, space="PSUM")
                    po_u = [
                        ps_out.tile([P, 512], F32, tag=f"po{u}", name=f"po{u}")
                        for u in range(8)
                    ]
                    for u in range(8):
                        tc_i, nt = u // 2, u % 2
                        nc.tensor.matmul(
                            po_u[u][:],
                            onesrow_b[:],
                            bo_b2[:, nt * 512 : (nt + 1) * 512],
                            start=True,
                            stop=False,
                        )
                        for g in range(NQT):
                            nc.tensor.matmul(
                                po_u[u][:],
                                ctxf[:, 0, g, tc_i * P : (tc_i + 1) * P],
                                wo_sb[:, 2 * g, nt * 512 : (nt + 1) * 512],
                                start=False,
                                stop=False,
                            )
                    with tc.tile_pool(name="out_pool", bufs=3) as out_pool:
                        for u in range(8):
                            tc_i, nt = u // 2, u % 2
                            for i, g in enumerate(range(NQT)):
                                nc.tensor.matmul(
                                    po_u[u][:],
                                    ctxf[:, 1, g, tc_i * P : (tc_i + 1) * P],
                                    wo_sb[:, 2 * g + 1, nt * 512 : (nt + 1) * 512],
                                    start=False,
                                    stop=(i == NQT - 1),
                                )
                            o_sb = out_pool.tile([P, 512], F32, tag="osb")
                            nc.scalar.copy(o_sb[:], po_u[u][:])
                            nc.sync.dma_start(
                                y_d[
                                    tc_i * P : (tc_i + 1) * P,
                                    nt * 512 : (nt + 1) * 512,
                                ],
                                o_sb[:],
                            )
                    ps_out.release()

                outp.release()

    nc.compile()
    return nc


_NC_CACHE = {}


def _get_nc():
    if "nc" not in _NC_CACHE:
        _NC_CACHE["nc"] = build()
    return _NC_CACHE["nc"]


def _to_bf16(a):
    return np.ascontiguousarray(a.astype(BF16_NP))


def _make_in_maps(x, Wq, bq, Wk, bk, Wv, bv, Wo, bo):
    x = np.asarray(x, np.float32)
    Wq, Wk, Wv, Wo = (np.asarray(a, np.float32) for a in (Wq, Wk, Wv, Wo))
    bq, bk, bv, bo = (np.asarray(a, np.float32) for a in (bq, bk, bv, bo))
    # xt[b][p, g, dc, t]: d = dc*128 + p, s = g*512 + t
    xts = [
        _to_bf16(
            x[b].T.reshape(DC, P, NQT, TB).transpose(1, 2, 0, 3)
        )
        for b in range(B)
    ]
    # wo[p, dc, n]: d = dc*128 + p
    wo_t = _to_bf16(Wo.reshape(DC, P, D).transpose(1, 0, 2))
    bo_r = np.ascontiguousarray(bo.reshape(1, D))
    in_maps = []
    for c in range(NC):
        b, g = c // 4, c % 4
        sl = slice(g * G, (g + 1) * G)
        gidx = (b * (D // 2) + np.arange(D // 2)).astype(np.int16)
        in_maps.append(
            {
                "xt": xts[b],
                "wq": _to_bf16(Wq[:, sl].reshape(DC, P, G).transpose(1, 0, 2)),
                "wk": _to_bf16(Wk[:, sl].reshape(DC, P, G).transpose(1, 0, 2)),
                "wv": _to_bf16(Wv[:, sl].reshape(DC, P, G).transpose(1, 0, 2)),
                "bq": np.ascontiguousarray(bq[sl].reshape(MC, P).T),
                "bk": np.ascontiguousarray(bk[sl].reshape(MC, P).T),
                "bv": np.ascontiguousarray(bv[sl].reshape(1, G)),
                "wo": wo_t,
                "bo": bo_r,
                "gidx": np.ascontiguousarray(
                    np.tile(gidx.reshape(D // 32, 16).T, (8, 1))
                ),
            }
        )
    return in_maps


def run(inputs, trace=False, tmpdir=None):
    """Run on 8 cores; returns (output [2,2048,1024], BassKernelResults)."""
    if trace:
        _install_ntff_hook()
    nc = _get_nc()
    in_maps = _make_in_maps(**inputs)
    res = bass_utils.run_bass_kernel_spmd(
        nc, in_maps, core_ids=list(range(NC)), trace=trace, tmpdir=tmpdir
    )
    out = np.empty((B, S, D), np.float32)
    for c in range(NC):
        b, g = c // 4, c % 4
        out[b, g * TB : (g + 1) * TB, :] = res.results[c]["y"]
    return out, res


def kernel(**inputs) -> np.ndarray:
    out, _ = run(inputs, trace=False)
    return out
